# revision 51
# baseline (speedup 1.0000x reference)
"""Sparse-attention kernel for trn2, data-parallel over batch on 8 NeuronCores.

Problem (hardcoded): x:(64,528,768) f32, Wq/Wk/Wv/Wp:(768,768), bp:(768,).
L = 528 tokens = 128 template/online-template tokens + 400 search tokens.
Queries 0:128 attend to keys 0:128; queries 128:528 attend to all 528 keys.
12 heads of dim 64, scale = 768**-0.5, out = softmax(qk^T*scale)v @ Wp + bp.

Sharding: batch 64 -> 8 cores x 8 batches. No collectives.

Device strategy (per core, per batch):
  - host pre-transposes x to xT (d-major); QK path in fp8e4m3, V path bf16.
  - QT/KT GEMMs run fp8 DoubleRow (2 k-rows per partition -> 0.5 PE
    cycles/out-column): host packs Wq/Wk with columns PERMUTED so that
    m-tile m holds the (m%2)-th 32-dim half of heads 4*(m//2)..4*(m//2)+3.
    The psum->SBUF copy is then partition-preserving into q8/k8
    [128, 3, 2, L] fp8: head h lives at partitions 32*(h%4):32*(h%4)+32,
    group h//4, with its two 32-dim k-halves adjacent in the free dim --
    exactly the [32p, 2, N] operand layout DoubleRow wants. Softmax is
    invariant to the (consistent) within-head dim permutation.
  - scores are computed transposed (S^T[t, l]) per head with fp8 DoubleRow
    (k=64 as 32 partitions x 2) at PE row base 32*(h%4).
  - fp8 on the q/k path only: score errors pass through softmax attenuated
    (scores*scale are O(0.1)), measured rel-err ~8e-3 vs the 2e-2 gate.
    V/EV/proj stay bf16 (fp8 there measures ~4e-2: v and p quantization
    errors hit the output at full relative strength).
  - V GEMM writes token-major V into a per-head 128-col stationary block:
      even head h: [ V(64) | ones(64) ]
      odd  head h: [ ones(64) | V(64) ]
    so one PV matmul per head emits O at the head's home lanes plus 64
    redundant copies of the softmax sums at the opposite lanes.
  - exp on ScalarE with the 1/sqrt(768) scale fused into the activation.
    Max-subtraction is skipped: scores are O(0.1), exp is exact there.
  - normalization: DVE reciprocal straight from the PSUM sums rows, DMA
    lane swap, DVE mul into OT (bf16).
  - V-block psum->SBUF copies run on gpsimd (Pool) to keep DVE off the
    critical path; projection copies on ScalarE.
  - projection GEMM -> Y^T, cast to bf16 in SBUF, DMA out; host transposes
    back and adds bp.
"""

import numpy as np
import ml_dtypes

import concourse.bass as bass
import concourse.mybir as mybir
import concourse.tile as tile
from concourse.bass_utils import run_bass_kernel_spmd

# ---- problem constants ------------------------------------------------------
B, L, D, H, DH = 64, 528, 768, 12, 64
NCORES = 8
BPC = B // NCORES          # batches per core
ND = D // 128              # 6 d-tiles
NG = ND // 2               # 3 fp8 head groups (4 heads each)
NT = (L + 127) // 128      # 5 token tiles (4x128 + 16)
TTAIL = L - 4 * 128        # 16
LA = 128                   # part-A queries (and keys)
LS = L - LA                # 400 part-B (search) queries
NP = H // 2                # 6 head pairs
SCALE = float(D) ** -0.5

BF = mybir.dt.bfloat16
F32 = mybir.dt.float32
F8 = mybir.dt.float8e4
F8NP = mybir.dt.np(mybir.dt.float8e4)
DR = mybir.MatmulPerfMode.DoubleRow


def _split_multi_waits(nc, max_waits=1):
    """walrus in this environment rejects instructions carrying more than
    one sync-wait command.  Tile's scheduler freely attaches several.  Hoist
    the extras onto dedicated same-engine NOPs emitted just before the
    instruction (engine streams execute a block in order, so the semantics
    are identical)."""
    n_split = 0
    for fn in nc.m.functions:
        for bb in fn.blocks:
            insts = list(bb.instructions)
            if not any(
                getattr(i, "sync_info", None) is not None
                and len(i.sync_info.on_wait) > max_waits
                for i in insts
            ):
                continue
            out = []
            for inst in insts:
                si = getattr(inst, "sync_info", None)
                if si is not None and len(si.on_wait) > max_waits:
                    waits = list(si.on_wait)
                    for w in waits[:-max_waits]:
                        nop = mybir.InstNoOp(
                            name=f"WS-{nc.next_id()}",
                            engine=inst.engine,
                            sync_info=mybir.SyncInfo(on_wait=[w], on_update=[]),
                            bass_nofuse=True,
                        )
                        nc.register_instruction(nop, overwrite=True)
                        out.append(nop)
                    inst.sync_info = mybir.SyncInfo(
                        on_wait=waits[-max_waits:], on_update=list(si.on_update)
                    )
                    n_split += 1
                out.append(inst)
            bb.instructions = out
    return n_split


def _tp(t):
    """token-partition count of token tile t (last tile is a 16-row tail)"""
    return 128 if t < NT - 1 else TTAIL


def qk_perm():
    """dout permutation for Wq/Wk: permuted column m*128+w holds original
    head-dim column h*64 + (m%2)*32 + (w%32) with h = 4*(m//2) + w//32."""
    p = np.empty(D, np.int64)
    for m in range(ND):
        g, i = m // 2, m % 2
        for w in range(128):
            h = 4 * g + w // 32
            p[m * 128 + w] = h * 64 + i * 32 + (w % 32)
    return p


TUNE = {
    "gap_budget": 700,     # PE-fill ns per score-tile stall
    "eta_act": 1100,       # ACT-fill ns at the eta slot
    "eta_gap": 500,        # extra PE-fill ns at the eta slot
    "proj_dve": 0,         # every Nth proj copy on DVE (0 = all ACT)
}


def build_bass(bpc=BPC, split_waits=True, repeat=1, pipeline=True,
               tune=None):
    tn = dict(TUNE, **(tune or {}))
    nc = bass.Bass()
    xt_ext = nc.declare_dram_parameter("xt", [bpc, D, L], BF, isOutput=False)
    xt8_ext = nc.declare_dram_parameter("xt8", [bpc, D, L], F8, isOutput=False)
    w_ext = {
        "wq": nc.declare_dram_parameter("wq", [D, D], F8, isOutput=False),
        "wk": nc.declare_dram_parameter("wk", [D, D], F8, isOutput=False),
        "wv": nc.declare_dram_parameter("wv", [D, D], BF, isOutput=False),
        "wp": nc.declare_dram_parameter("wp", [D, D], BF, isOutput=False),
    }
    yt_ext = nc.declare_dram_parameter("yt", [bpc, D, L], BF, isOutput=True)
    nbody = repeat * bpc

    with tile.TileContext(nc) as tc:
        with (
            tc.tile_pool(name="const", bufs=1) as constp,
            tc.tile_pool(name="xt", bufs=3) as xtp,
            tc.tile_pool(name="x8", bufs=3) as x8p,
            tc.tile_pool(name="qt", bufs=3) as qtp,
            tc.tile_pool(name="kt", bufs=3) as ktp,
            tc.tile_pool(name="et", bufs=4) as etp,
            tc.tile_pool(name="eta", bufs=4) as etap,
            tc.tile_pool(name="ot", bufs=3) as otp,
            tc.tile_pool(name="rst", bufs=6) as rstp,
            tc.tile_pool(name="rbc", bufs=6) as rbcp,
            tc.tile_pool(name="yst", bufs=6) as ystp,
            # PSUM budget: 8 banks, statically reserved per pool:
            # mm 1-bank x2, st 2-bank x1, o 2-bank x1, a 1, y 1
            tc.tile_pool(name="ps_mm", bufs=2, space="PSUM") as psmm,
            tc.tile_pool(name="ps_st", bufs=1, space="PSUM") as psst,
            tc.tile_pool(name="ps_o", bufs=1, space="PSUM") as pso,
            tc.tile_pool(name="ps_a", bufs=1, space="PSUM") as psa,
            tc.tile_pool(name="ps_y", bufs=1, space="PSUM") as psy,
        ):
            # ---- weights, k-tile-major: [128, k_tile, dout].  Only wq is
            # loaded up front; the rest are issued after xt(0) so the first
            # QT GEMM isn't queued behind the weight DMA.
            w_sb = {}
            for n, dt_ in (("wq", F8), ("wk", F8), ("wv", BF), ("wp", BF)):
                w_sb[n] = constp.tile([128, ND, D], dt_, tag=n, name=n)

            def load_w(n):
                # alternate the two HWDGE queues (SP, ACT) so weight DMAs
                # run two-wide instead of serializing on one queue
                wr = w_ext[n].rearrange("(n p) m -> p n m", p=128)
                for k in range(ND):
                    eng = nc.sync if k % 2 == 0 else nc.scalar
                    eng.dma_start(w_sb[n][:, k, :], wr[:, k, :])

            load_w("wq")

            # ---- static V-block tiles (double buffered manually) ------------
            # layout [128 tokens, NT, NP, parity, 128]:
            #   parity 0 (even head): cols 0:64 V,    cols 64:128 ones
            #   parity 1 (odd head):  cols 0:64 ones, cols 64:128 V
            vz_tiles = []
            for i in range(2):
                v = constp.tile([128, NT, NP, 2, 128], BF, tag=f"vz{i}")
                nc.gpsimd.memset(v[:, :, :, 0, 64:128], 1.0)
                nc.gpsimd.memset(v[:, :, :, 1, 0:64], 1.0)
                vz_tiles.append(v)

            # warm the ACT exp table during the weight-DMA window so the
            # table load isn't on the first attention pair's path
            warm = constp.tile([1, 1], F32, tag="warm")
            nc.scalar.activation(
                warm[:], vz_tiles[0][0:1, 0, 0, 0, 64:65],
                mybir.ActivationFunctionType.Exp, scale=1.0,
            )

            tiles = {}

            def unit(fn, pe):
                return {"fn": fn, "pe": pe}

            def gemm_units(rb, first=False):
                """Closures emitting the QKV GEMMs of body rb, unit-granular
                so they can be interleaved into the previous body's
                attention to keep PE busy during its ACT-paced stretch."""
                b = rb % bpc
                units = []

                def load_x():
                    # body 0's x loads carry no tile-reuse waits, so they can
                    # ride the ACT hwdge queue without stalling it; later
                    # bodies stay on SP (their WAR waits would block ACT)
                    eng = nc.scalar if rb == 0 else nc.sync
                    xt = xtp.tile([128, ND, L], BF, tag="xt")
                    eng.dma_start(
                        xt[:], xt_ext[b].rearrange("(n p) m -> p n m", p=128)
                    )
                    x8 = x8p.tile([128, ND, L], F8, tag="x8")
                    eng.dma_start(
                        x8[:], xt8_ext[b].rearrange("(n p) m -> p n m", p=128)
                    )
                    tiles[rb] = {
                        "xt": xt,
                        "x8": x8,
                        # q8/k8: [128p, group, khalf, L] fp8; head h at
                        # partitions 32*(h%4)+/-, group h//4
                        "qt": qtp.tile([128, NG, 2, L], F8, tag="qt", name="qt"),
                        "kt": ktp.tile([128, NG, 2, L], F8, tag="kt", name="kt"),
                        "vz": vz_tiles[rb % 2],
                    }

                units.append(unit(load_x, 0))

                def qk_unit(wname, dname, m, c):
                    def emit():
                        t = tiles[rb]
                        ps = psmm.tile([128, 512], F32, tag="mm")
                        for k2 in range(ND // 2):
                            nc.tensor.matmul(
                                ps[:, 0:264],
                                w_sb[wname][:, 2 * k2:2 * k2 + 2,
                                            m * 128:(m + 1) * 128],
                                t["x8"][:, 2 * k2:2 * k2 + 2,
                                        c * 264:(c + 1) * 264],
                                start=(k2 == 0), stop=(k2 == ND // 2 - 1),
                                perf_mode=DR,
                            )
                        nc.vector.tensor_copy(
                            t[dname][:, m // 2, m % 2, c * 264:(c + 1) * 264],
                            ps[:, 0:264],
                        )
                    return emit

                def v_unit(t_, c):
                    def emit():
                        t = tiles[rb]
                        tp = _tp(t_)
                        ps = psmm.tile([128, 512], F32, tag="mm")
                        for k in range(ND):
                            nc.tensor.matmul(
                                ps[0:tp, 0:384],
                                t["xt"][:, k, t_ * 128:t_ * 128 + tp],
                                w_sb["wv"][:, k, c * 384:(c + 1) * 384],
                                start=(k == 0), stop=(k == ND - 1),
                            )
                        p0 = 3 * c
                        chunk = ps[0:tp, 0:384].rearrange(
                            "p (pr q n) -> p pr q n", pr=3, q=2
                        )
                        nc.vector.tensor_copy(
                            t["vz"][0:tp, t_, p0:p0 + 3, 0, 0:64],
                            chunk[:, :, 0, :],
                        )
                        nc.scalar.copy(
                            t["vz"][0:tp, t_, p0:p0 + 3, 1, 64:128],
                            chunk[:, :, 1, :],
                        )
                    return emit

                if first:
                    for m in range(ND):
                        for c in range(2):
                            units.append(unit(qk_unit("wq", "qt", m, c), 165))
                    units.append(unit(lambda: (load_w("wk"), load_w("wv"),
                                               load_w("wp")), 0))
                    for m in range(ND):
                        for c in range(2):
                            units.append(unit(qk_unit("wk", "kt", m, c), 165))
                else:
                    for m in range(ND):
                        for c in range(2):
                            units.append(unit(qk_unit("wq", "qt", m, c), 165))
                            units.append(unit(qk_unit("wk", "kt", m, c), 165))
                for t_ in range(NT):
                    for c in range(2):
                        units.append(unit(v_unit(t_, c), 960))
                return units

            def proj_units(rb, alt=False):
                b = rb % bpc
                units = []

                ysts = {}

                def y_unit(m, c):
                    # in the epilogue the EV part-A bank is free; alternate
                    # projection accumulators across it to overlap the
                    # ScalarE copy of unit u with the matmuls of unit u+1
                    pool = psa if (alt and (2 * m + c) % 2 == 1) else psy

                    def emit():
                        ot = tiles[rb]["ot"]
                        yp = pool.tile([128, 264], F32, tag="y" if pool is psy else "a")
                        for k in range(ND):
                            nc.tensor.matmul(
                                yp[:],
                                w_sb["wp"][:, k, m * 128:(m + 1) * 128],
                                ot[:, k, c * 264:(c + 1) * 264],
                                start=(k == 0), stop=(k == ND - 1),
                            )
                        if m not in ysts:
                            ysts[m] = ystp.tile(
                                [128, L], BF, tag="yst", name="yst"
                            )
                        yst = ysts[m]
                        # split the projection copies ACT/DVE to balance
                        pd = tn["proj_dve"]
                        if pd and (2 * m + c) % 3 == pd - 1 and not alt:
                            nc.vector.tensor_copy(
                                yst[:, c * 264:(c + 1) * 264], yp[:]
                            )
                        else:
                            nc.scalar.copy(
                                yst[:, c * 264:(c + 1) * 264], yp[:]
                            )
                        if c == 1:
                            # yt store waits only on the ACT copies above --
                            # same-engine program order -- so it can ride the
                            # ACT hwdge queue, keeping SP free for the
                            # latency-critical rbc lane swaps
                            nc.scalar.dma_start(
                                yt_ext[b, m * 128:(m + 1) * 128, :],
                                yst[:],
                            )
                    return emit

                for m in range(ND):
                    for c in range(2):
                        units.append(unit(y_unit(m, c), 660))
                return units

            def attn_pair(rb, p, fill=()):
                t = tiles[rb]
                qt, kt, vz, ot = t["qt"], t["kt"], t["vz"], t["ot"]
                # part B scores S^T[t, l], one fp8 DoubleRow matmul per head
                # (k=64 as 32 partitions x 2), at PE row base 32*(h%4).
                et = etp.tile([128, NT, 2, LS], BF, tag="et")
                heads = (2 * p, 2 * p + 1)
                hb = [32 * (h % 4) for h in heads]
                hg = [h // 4 for h in heads]
                for t_ in range(NT):
                    tp = _tp(t_)
                    stp = psst.tile([128, 2, 512], F32, tag="st")
                    for j in range(2):
                        nc.tensor.matmul(
                            stp[0:tp, j, 0:LS],
                            kt[hb[j]:hb[j] + 32, hg[j], :,
                               t_ * 128:t_ * 128 + tp],
                            qt[hb[j]:hb[j] + 32, hg[j], :, LA:L],
                            perf_mode=DR,
                            tile_position=(hb[j], 0),
                        )
                    nc.scalar.activation(
                        et[0:tp, t_, :, :], stp[0:tp, :, 0:LS],
                        mybir.ActivationFunctionType.Exp, scale=SCALE,
                    )

                # part A scores (keys 0:128, queries 0:128)
                sta = psst.tile([128, 2, 512], F32, tag="st")
                eta = etap.tile([128, 2, LA], BF, tag="eta")
                for j in range(2):
                    nc.tensor.matmul(
                        sta[:, j, 0:LA],
                        kt[hb[j]:hb[j] + 32, hg[j], :, 0:LA],
                        qt[hb[j]:hb[j] + 32, hg[j], :, 0:LA],
                        perf_mode=DR,
                        tile_position=(hb[j], 0),
                    )
                nc.scalar.activation(
                    eta[:], sta[:, :, 0:LA],
                    mybir.ActivationFunctionType.Exp, scale=SCALE,
                )

                for u in fill:
                    u["fn"]()

                # EV part B: accumulate over token tiles.
                # even head (j=0): O rows 0:64, sums copies rows 64:128
                # odd  head (j=1): sums copies rows 0:64, O rows 64:128
                ops = pso.tile([128, 2, 512], F32, tag="o")
                for j in range(2):
                    for t_ in range(NT):
                        tp = _tp(t_)
                        nc.tensor.matmul(
                            ops[:, j, 0:LS],
                            vz[0:tp, t_, p, j, :],
                            et[0:tp, t_, j, :],
                            start=(t_ == 0), stop=(t_ == NT - 1),
                        )
                # EV part A (keys tile 0 only)
                oa = psa.tile([128, 512], F32, tag="a")
                nc.tensor.matmul(oa[:, 0:LA], vz[:, 0, p, 0, :], eta[:, 0, :])
                nc.tensor.matmul(
                    oa[:, LA:2 * LA], vz[:, 0, p, 1, :], eta[:, 1, :]
                )

                # reciprocal of the sums straight from PSUM (the ones-columns
                # replicated the sums across 64 lanes)
                rst = rstp.tile([128, L], F32, tag="rst")
                nc.vector.reciprocal(rst[64:128, LA:L], ops[64:128, 0, 0:LS])
                nc.vector.reciprocal(rst[0:64, LA:L], ops[0:64, 1, 0:LS])
                nc.vector.reciprocal(rst[64:128, 0:LA], oa[64:128, 0:LA])
                nc.vector.reciprocal(rst[0:64, 0:LA], oa[0:64, LA:2 * LA])

                # swap the lane halves so each head's recip lands on its home
                # lanes (plain strided SBUF->SBUF DMA)
                rbc = rbcp.tile([128, L], F32, tag="rbc")
                nc.sync.dma_start(rbc[0:64, :], rst[64:128, :])
                nc.sync.dma_start(rbc[64:128, :], rst[0:64, :])

                # scale into the merged d-major OT tile (bf16)
                nc.vector.tensor_mul(
                    ot[0:64, p, LA:L], ops[0:64, 0, 0:LS], rbc[0:64, LA:L]
                )
                nc.vector.tensor_mul(
                    ot[64:128, p, LA:L], ops[64:128, 1, 0:LS],
                    rbc[64:128, LA:L],
                )
                nc.vector.tensor_mul(
                    ot[0:64, p, 0:LA], oa[0:64, 0:LA], rbc[0:64, 0:LA]
                )
                nc.vector.tensor_mul(
                    ot[64:128, p, 0:LA], oa[64:128, LA:2 * LA],
                    rbc[64:128, 0:LA],
                )

            def slice_units(units, p):
                n = len(units)
                return units[p * n // NP:(p + 1) * n // NP]

            # ---- software pipeline: attention(rb) interleaved with
            # GEMM(rb+1) and projection(rb-1) --------------------------------
            for u in gemm_units(0, first=True):
                u["fn"]()
            for rb in range(nbody):
                tiles[rb]["ot"] = otp.tile([128, ND, L], BF, tag="ot",
                                           name="ot")
                gu = gemm_units(rb + 1) if rb + 1 < nbody else []
                pu = proj_units(rb - 1) if rb >= 1 else []
                if pipeline:
                    for p in range(NP):
                        su = slice_units(gu, p)
                        pv = slice_units(pu, p)
                        if su:
                            h = len(su) // 2
                            attn_pair(rb, p, fill=su[:h])
                            rest = su[h:] + pv
                        else:
                            # last body: no next-batch GEMMs; put the proj
                            # filler at the mid-pair stall point instead
                            h = len(pv) // 2
                            attn_pair(rb, p, fill=pv[:h])
                            rest = pv[h:]
                        for u in rest:
                            u["fn"]()
                else:
                    for p in range(NP):
                        attn_pair(rb, p)
                    for u in gu + pu:
                        u["fn"]()
                if rb >= 2:
                    tiles.pop(rb - 2, None)
            for u in proj_units(nbody - 1, alt=True):
                u["fn"]()

    if split_waits:
        _split_multi_waits(nc, max_waits=int(__import__('os').environ.get('MAXW', '1')))
    return nc


_CACHE = {}


def _get_bass():
    if "nc" not in _CACHE:
        _CACHE["nc"] = build_bass()
    return _CACHE["nc"]


def make_in_maps(x, Wq, Wk, Wv, Wp):
    """Per-core input dicts (host-side sharding + packing)."""
    x = np.asarray(x, np.float32)
    assert x.shape == (B, L, D), x.shape
    xt_f = np.ascontiguousarray(
        x.reshape(NCORES, BPC, L, D).transpose(0, 1, 3, 2)
    )
    xt = xt_f.astype(ml_dtypes.bfloat16)
    xt8 = xt_f.astype(F8NP)
    perm = qk_perm()
    wq = np.ascontiguousarray(
        np.asarray(Wq, np.float32)[:, perm]).astype(F8NP)
    wk = np.ascontiguousarray(
        np.asarray(Wk, np.float32)[:, perm]).astype(F8NP)
    wv = np.ascontiguousarray(np.asarray(Wv, np.float32)).astype(
        ml_dtypes.bfloat16)
    wp = np.ascontiguousarray(np.asarray(Wp, np.float32)).astype(
        ml_dtypes.bfloat16)
    return [
        {"xt": xt[i], "xt8": xt8[i], "wq": wq, "wk": wk, "wv": wv, "wp": wp}
        for i in range(NCORES)
    ]


def kernel(x, Wq, Wk, Wv, Wp, bp, t_h=8, t_w=8, s_h=20, s_w=20, _trace=False):
    assert int(t_h) * int(t_w) == 64 and int(s_h) * int(s_w) == 400

    nc = _get_bass()
    in_maps = make_in_maps(x, Wq, Wk, Wv, Wp)
    res = run_bass_kernel_spmd(
        nc, in_maps, core_ids=list(range(NCORES)), trace=_trace
    )
    y = np.stack(
        [np.asarray(res.results[i]["yt"], np.float32) for i in range(NCORES)]
    )
    y = y.transpose(0, 1, 3, 2).reshape(B, L, D)
    y = y + np.asarray(bp, np.float32)[None, None, :]
    if _trace:
        _CACHE["last_result"] = res
    return y.astype(np.float32)
